# revision 5
# baseline (speedup 1.0000x reference)
"""CRF loss (nn_CRFLayer) on 8 Trainium2 NeuronCores.

Strategy (pure data parallel over batch, per sharding hint):
  B=4096 split into 8 shards of 512. Per core, 512 sequences are packed as
  4 groups x 128 partitions; state v[b', 32g+t] = exp(alpha - c) is kept in
  exp-domain with a per-(b,g) normalizer c, so the per-step logsumexp becomes
  a 128x132 matmul with the constant block-diagonal matrix exp(transitions)^T
  (plus 4 block-ones columns that yield the per-group sums for free).
  Gold score: emission gather via onehot compare + fused multiply-reduce on
  chunk-resident feats; transition pair values are host-marshalled (pure index
  lookup) and summed on device. Loss partial per core -> host mean.
"""
import sys
import numpy as np

sys.path.insert(0, "/opt/trn_rl_repo")

B, S, T = 4096, 512, 32
START, STOP = 30, 31
NEG = -10000.0
NCORES = 8
BC = B // NCORES          # 512 sequences per core
G = 4                     # groups per core
P = 128                   # partitions
CH = 64                   # steps per feats chunk
NCH = S // CH
RENORM = 4

_compiled = None


def _build_bass():
    import concourse.bass as bass
    import concourse.mybir as mybir
    from concourse.tile import TileContext

    f32 = mybir.dt.float32
    AF = mybir.ActivationFunctionType
    ALU = mybir.AluOpType
    AX = mybir.AxisListType

    nc = bass.Bass()
    feats_h = nc.dram_tensor("feats", [BC, S, T], f32, kind="ExternalInput")
    mext_h = nc.dram_tensor("m_ext", [P, P + G], f32, kind="ExternalInput")
    ident_h = nc.dram_tensor("ident", [P, P], f32, kind="ExternalInput")
    tagsf_h = nc.dram_tensor("tags_eff", [P, G, S], f32, kind="ExternalInput")
    pair_h = nc.dram_tensor("pairval_eff", [P, G, S], f32, kind="ExternalInput")
    u8 = mybir.dt.uint8
    maskl_h = nc.dram_tensor("maskL", [P, S + 1, G], u8, kind="ExternalInput")
    tpos_h = nc.dram_tensor("tpos", [P, T], f32, kind="ExternalInput")
    loss_h = nc.dram_tensor("loss_part", [1, 1], f32, kind="ExternalOutput")

    with TileContext(nc) as tc:
        with (
            tc.tile_pool(name="singles", bufs=1) as singles,
            tc.tile_pool(name="fpool", bufs=2) as fpool,
            tc.tile_pool(name="state", bufs=3) as state,
            tc.tile_pool(name="small", bufs=4) as small,
            tc.tile_pool(name="work", bufs=2) as work,
            tc.tile_pool(name="ps_t", bufs=2, space="PSUM") as ps_t,
            tc.tile_pool(name="ps_s", bufs=2, space="PSUM") as ps_s,
            tc.tile_pool(name="ps_f", bufs=1, space="PSUM") as ps_f,
        ):
            # ---- static loads ----
            m_sb = singles.tile([P, P + G], f32)
            nc.sync.dma_start(out=m_sb[:], in_=mext_h[:])
            id_sb = singles.tile([P, P], f32)
            nc.sync.dma_start(out=id_sb[:], in_=ident_h[:])
            tags_sb = singles.tile([P, G, S], f32)
            nc.sync.dma_start(out=tags_sb[:], in_=tagsf_h[:])
            pair_sb = singles.tile([P, G, S], f32)
            nc.sync.dma_start(out=pair_sb[:], in_=pair_h[:])
            maskl_sb = singles.tile([P, S + 1, G], u8)
            nc.sync.dma_start(out=maskl_sb[:], in_=maskl_h[:])
            tpos_sb = singles.tile([P, T], f32)
            nc.sync.dma_start(out=tpos_sb[:], in_=tpos_h[:])

            # ---- state init ----
            v = state.tile([P, P], f32, tag="v")
            nc.vector.memset(v[:], 0.0)
            nc.vector.memset(v.rearrange("p (g t) -> p g t", g=G)[:, :, START], 1.0)
            c = state.tile([P, G], f32, tag="c")
            nc.vector.memset(c[:], 0.0)
            fwd_sum = singles.tile([P, G], f32)
            nc.vector.memset(fwd_sum[:], 0.0)
            fwd_c = singles.tile([P, G], f32)
            nc.vector.memset(fwd_c[:], 0.0)
            em_parts = singles.tile([P, NCH, G], f32)

            feats_r = feats_h.rearrange("(g p) s t -> p g s t", p=P)

            for k in range(NCH):
                # chunk DMA: [P, G, CH, T]
                fk = fpool.tile([P, G, CH, T], f32, tag="fk")
                nc.sync.dma_start(out=fk[:], in_=feats_r[:, :, k * CH:(k + 1) * CH, :])

                # gold emission for this chunk (off critical path):
                # onehot = (tpos == tag) ; em_part[g] = sum(onehot * F)
                oh = work.tile([P, G, CH, T], f32, tag="oh")
                tpos_b = bass.AP(
                    tensor=tpos_sb.tensor, offset=tpos_sb.offset,
                    ap=[tpos_sb.ap[0], [0, G], [0, CH], tpos_sb.ap[1]],
                )
                tags_ch = tags_sb[:, :, k * CH:(k + 1) * CH]
                tags_b = bass.AP(
                    tensor=tags_ch.tensor, offset=tags_ch.offset,
                    ap=[*tags_ch.ap, [0, T]],
                )
                nc.vector.tensor_tensor(out=oh[:], in0=tpos_b, in1=tags_b,
                                        op=ALU.is_equal)
                junk = work.tile([P, CH * T], f32, tag="junk")
                for g in range(G):
                    nc.vector.scalar_tensor_tensor(
                        out=junk[:],
                        in0=oh[:, g, :, :].rearrange("p a b -> p (a b)"),
                        scalar=1.0,
                        in1=fk[:, g, :, :].rearrange("p a b -> p (a b)"),
                        op0=ALU.mult, op1=ALU.mult,
                        accum_out=em_parts[:, k, g:g + 1],
                    )

                for sl in range(CH):
                    s = k * CH + sl
                    # transpose v -> [(g,frm), b']  (PSUM)
                    vt_ps = ps_t.tile([P, P], f32, tag="vt")
                    nc.tensor.transpose(vt_ps[:], v[:], id_sb[:])
                    vt_sb = state.tile([P, P], f32, tag="vts")
                    nc.scalar.copy(vt_sb[:], vt_ps[:])
                    # S_ext = vT^T @ [M_bd | ones_bd]: [P, 128+4]
                    s_ps = ps_s.tile([P, P + G], f32, tag="sx")
                    nc.tensor.matmul(s_ps[:], lhsT=vt_sb[:], rhs=m_sb[:],
                                     start=True, stop=True)
                    # exp of emissions for this step
                    ef = state.tile([P, G, T], f32, tag="ef")
                    nc.scalar.activation(ef[:], fk[:, :, sl, :], AF.Exp)
                    # extraction of lattice position s (before update)
                    nc.vector.copy_predicated(fwd_sum[:], maskl_sb[:, s, :],
                                              s_ps[:, P:P + G])
                    nc.vector.copy_predicated(fwd_c[:], maskl_sb[:, s, :], c[:])
                    # v_new = S * exp(F)
                    v_new = state.tile([P, P], f32, tag="v")
                    nc.vector.tensor_mul(
                        v_new.rearrange("p (g t) -> p g t", g=G),
                        s_ps[:, 0:P].rearrange("p (g t) -> p g t", g=G),
                        ef[:],
                    )
                    v = v_new
                    if s % RENORM == RENORM - 1:
                        r4 = small.tile([P, G], f32, tag="r4")
                        nc.vector.reciprocal(r4[:], s_ps[:, P:P + G])
                        lnr = small.tile([P, G], f32, tag="lnr")
                        nc.scalar.activation(lnr[:], s_ps[:, P:P + G], AF.Ln)
                        v2 = state.tile([P, P], f32, tag="v")
                        r4_b = bass.AP(tensor=r4.tensor, offset=r4.offset,
                                       ap=[*r4.ap, [0, T]])
                        nc.vector.tensor_tensor(
                            out=v2.rearrange("p (g t) -> p g t", g=G),
                            in0=v.rearrange("p (g t) -> p g t", g=G),
                            in1=r4_b, op=ALU.mult)
                        c_new = state.tile([P, G], f32, tag="c")
                        nc.vector.tensor_add(c_new[:], c[:], lnr[:])
                        v, c = v2, c_new

            # ---- epilogue: lattice position S ----
            sumv = small.tile([P, G], f32, tag="sumv")
            nc.vector.tensor_reduce(sumv[:], v.rearrange("p (g t) -> p g t", g=G),
                                    axis=AX.X, op=ALU.add)
            nc.vector.copy_predicated(fwd_sum[:], maskl_sb[:, S, :], sumv[:])
            nc.vector.copy_predicated(fwd_c[:], maskl_sb[:, S, :], c[:])

            # fwd = ln(fwd_sum) + fwd_c   (= lse(alpha_len); NEG dropped, cancels gold's)
            lnf = small.tile([P, G], f32, tag="lnf")
            nc.scalar.activation(lnf[:], fwd_sum[:], AF.Ln)
            fwd = small.tile([P, G], f32, tag="fwd")
            nc.vector.tensor_add(fwd[:], lnf[:], fwd_c[:])

            # gold sums
            em4 = small.tile([P, G], f32, tag="em4")
            nc.vector.tensor_reduce(
                em4[:],
                bass.AP(tensor=em_parts.tensor, offset=em_parts.offset,
                        ap=[em_parts.ap[0], [1, G], [G, NCH]]),
                axis=AX.X, op=ALU.add)
            tr4 = small.tile([P, G], f32, tag="tr4")
            nc.vector.tensor_reduce(tr4[:], pair_sb[:], axis=AX.X, op=ALU.add)

            loss4 = small.tile([P, G], f32, tag="loss4")
            nc.vector.tensor_sub(loss4[:], fwd[:], em4[:])
            nc.vector.tensor_sub(loss4[:], loss4[:], tr4[:])

            # partition-sum: [P,G] -> [G,1] -> [1,1]
            ones_p = singles.tile([P, 1], f32)
            nc.vector.memset(ones_p[:], 1.0)
            ps1 = ps_f.tile([G, 1], f32, tag="ps1")
            nc.tensor.matmul(ps1[:], lhsT=loss4[:], rhs=ones_p[:],
                             start=True, stop=True)
            ps1_sb = small.tile([G, 1], f32, tag="ps1s")
            nc.scalar.copy(ps1_sb[:], ps1[:])
            ps2 = ps_f.tile([1, 1], f32, tag="ps2")
            nc.tensor.matmul(ps2[:], lhsT=ps1_sb[:], rhs=ones_p[0:G, :],
                             start=True, stop=True)
            out_sb = small.tile([1, 1], f32, tag="outs")
            nc.scalar.copy(out_sb[:], ps2[:])
            nc.sync.dma_start(out=loss_h[:], in_=out_sb[:])

    return nc


def _host_inputs(feats, tags, lengths, transitions):
    feats = np.ascontiguousarray(np.asarray(feats, np.float32))
    tags = np.asarray(tags).astype(np.int64)
    lengths = np.asarray(lengths).astype(np.int64)
    transitions = np.asarray(transitions, np.float32)

    # block-diag exp(trans)^T plus ones columns
    m = np.exp(transitions.T.astype(np.float64)).astype(np.float32)  # [frm, to]
    m_ext = np.zeros((P, P + G), np.float32)
    for g in range(G):
        m_ext[g * T:(g + 1) * T, g * T:(g + 1) * T] = m
        m_ext[g * T:(g + 1) * T, P + g] = 1.0
    ident = np.eye(P, dtype=np.float32)
    tpos = np.broadcast_to(np.arange(T, dtype=np.float32), (P, T)).copy()

    flat = transitions.reshape(-1)
    tags_prev = np.concatenate(
        [np.full((B, 1), START, np.int64), tags[:, :-1]], axis=1)
    pairval = flat[(tags * T + tags_prev).reshape(-1)].reshape(B, S)
    smask = np.arange(S)[None, :] < lengths[:, None]
    pairval_eff = np.where(smask, pairval, 0.0).astype(np.float32)
    tags_eff = np.where(smask, tags, 127).astype(np.float32)

    per_core = []
    for core in range(NCORES):
        sl = slice(core * BC, (core + 1) * BC)
        f_c = feats[sl]
        te_c = tags_eff[sl].reshape(G, P, S).transpose(1, 0, 2)
        pv_c = pairval_eff[sl].reshape(G, P, S).transpose(1, 0, 2)
        len_c = lengths[sl].reshape(G, P).T  # [P, G]
        maskl = np.zeros((P, S + 1, G), np.uint8)
        pp, gg = np.meshgrid(np.arange(P), np.arange(G), indexing="ij")
        maskl[pp, len_c, gg] = 1
        per_core.append({
            "feats": f_c,
            "m_ext": m_ext,
            "ident": ident,
            "tags_eff": np.ascontiguousarray(te_c),
            "pairval_eff": np.ascontiguousarray(pv_c),
            "maskL": maskl,
            "tpos": tpos,
        })
    return per_core


def kernel(feats, tags, lengths, transitions):
    global _compiled
    from concourse.bass_utils import run_bass_kernel_spmd
    import waitfix_embedded  # noqa: F401  (installs on import)

    if _compiled is None:
        _compiled = _build_bass()
    nc = _compiled
    in_maps = _host_inputs(feats, tags, lengths, transitions)
    res = run_bass_kernel_spmd(nc, in_maps, core_ids=list(range(NCORES)))
    total = np.float64(0.0)
    for r in res.results:
        total += np.float64(r["loss_part"][0, 0])
    return np.float32(total / B)


# ---- embedded waitfix module (kernel.py must be self-contained) ----
import types as _types  # noqa: E402

_wf_src = '''
import json

MAX_WAITS = 1

def split_sync_waits(bir_bytes, max_waits=MAX_WAITS):
    bir = json.loads(bir_bytes)
    n_split = 0
    for fn in bir["functions"]:
        for blk in fn["blocks"]:
            out = []
            for inst in blk["instructions"]:
                si = inst.get("sync_info")
                waits = (si or {}).get("on_wait") or []
                if len(waits) > max_waits:
                    k = 0
                    while len(waits) > max_waits:
                        chunk, waits = waits[:max_waits], waits[max_waits:]
                        out.append({
                            "debug": inst.get("debug", 0),
                            "engine": inst["engine"],
                            "ins": [], "is_reset_sema": False,
                            "name": inst["name"] + "-wsplit%d" % k,
                            "opcode": "NoOp", "outs": [],
                            "sync_info": {"on_update": [], "on_wait": chunk},
                        })
                        k += 1
                    si["on_wait"] = waits
                    n_split += 1
                out.append(inst)
            blk["instructions"] = out
    return json.dumps(bir).encode()

def install():
    import concourse.bass2jax as bass2jax
    if getattr(bass2jax, "_waitfix_installed", False):
        return
    orig = bass2jax.compile_bir_kernel
    def patched(bir_json, tmpdir, neff_name="file.neff"):
        return orig(split_sync_waits(bir_json), tmpdir, neff_name)
    bass2jax.compile_bir_kernel = patched
    bass2jax._waitfix_installed = True

install()
'''
if "waitfix_embedded" not in sys.modules:
    _mod = _types.ModuleType("waitfix_embedded")
    exec(_wf_src, _mod.__dict__)
    sys.modules["waitfix_embedded"] = _mod


if __name__ == "__main__":
    import refcache
    inputs, exp = refcache.load()
    out = kernel(**inputs)
    rel = abs(float(out) - float(exp)) / max(abs(float(exp)), 1e-9)
    print("kernel:", out, "expected:", exp, "rel err:", rel)
